# revision 4
# baseline (speedup 1.0000x reference)
"""BFP activation quantization kernel for 8 TRN2 NeuronCores.

Reference semantics (per (n,h,w) block over C=256 channels):
    max_abs = max_c |x|
    e such that max_abs = m * 2^e, m in [0.5, 1)   (frexp)
    delta = 2^(e-3)
    q = trunc(x / delta) * delta

Sharding: pure data-parallel over N (batch): 64 images -> 8 per core.

Per-core implementation (layout [c -> partitions, hw -> free], fully
contiguous DMA):
  - gpsimd.partition_all_reduce(absmax) gives per-column |max| over the 128
    partitions, broadcast to all partitions; one vector max merges the two
    C halves.
  - The scale factors are pure exponent-bit arithmetic on int32 views:
        eb = bits(max_abs) & 0x7f800000
        bits(-delta) = eb + 0x7f000000   == int32 value (eb - 0x81000000)
        bits(-recip) = 2^32 - eb         == int32 value (-eb)
    (computed on ScalarE as exact fp32-valued integer ops)
  - trunc(x * recip) with only round-to-nearest-even converters:
        x |= 1            (odd mantissa -> t = x*recip is never an exact
                           integer, so RN ties at k-0.5 never bite)
        t  = x * -recip   (exact; u = -t = x*recip)
        p  = cvt_i32(Relu(-t - c)), n = cvt_i32(Relu(t - c)) on ScalarE
                           with c = 0.5 - 2^-25  -> trunc(u) = p - n
        q  = (p - n) * delta
    The Relu pair does the half-step bias, clamp, and int conversion on the
    otherwise-idle ScalarE, keeping VectorE to 5 wide passes. Verified
    bit-exact vs the reference on HW over exhaustive mantissa sweeps and
    full-tensor runs (0/51.4M mismatches).
"""

import sys

for _p in ("/opt/trn_rl_repo", "/root/.axon_site/_ro/trn_rl_repo"):
    if _p not in sys.path:
        sys.path.append(_p)

import numpy as np

N, C, H, W = 64, 256, 56, 56
HW = H * W  # 3136
NCORES = 8
NPC = N // NCORES  # images per core
F = 3136  # free-dim chunk = full hw row per image
NCHUNK = HW // F


def _i32(v):
    v &= 0xFFFFFFFF
    return v - (1 << 32) if v >= (1 << 31) else v


_cache = {}


def _build(repeat=1):
    if ("nc", repeat) in _cache:
        return _cache[("nc", repeat)]

    import concourse.bacc as bacc
    import concourse.mybir as mybir
    import concourse.tile as tile
    from concourse import bass_isa

    dt = mybir.dt
    op = mybir.AluOpType

    nc = bacc.Bacc(
        "TRN2",
        target_bir_lowering=False,
        debug=False,
        enable_asserts=False,
        num_devices=NCORES,
    )
    x_d = nc.dram_tensor("x", [NPC, C, HW], dt.float32, kind="ExternalInput").ap()
    y_d = nc.dram_tensor("y", [NPC, C, HW], dt.float32, kind="ExternalOutput").ap()

    with tile.TileContext(nc) as tc:
        with (
            tc.tile_pool(name="big", bufs=2) as big,
            tc.tile_pool(name="small", bufs=1) as small,
            tc.tile_pool(name="consts", bufs=1) as consts,
        ):
            bias = consts.tile([128, 1], dt.float32)  # -(0.5 - 2^-25)
            nc.vector.memset(bias[:], -0.4999999701976776123046875)
            dbias = consts.tile([128, 1], dt.float32)  # delta: eb - 0x01000000
            nc.vector.memset(dbias[:], -float(0x01000000))
            for nn in range(NPC * repeat):
                n = nn % NPC
                xt = big.tile([128, 2 * F], dt.float32, tag="xt")
                nc.sync.dma_start(out=xt[:, 0:F], in_=x_d[n, 0:128, :])
                nc.sync.dma_start(out=xt[:, F : 2 * F], in_=x_d[n, 128:256, :])

                mxt = big.tile([128, 2 * F], dt.float32, tag="mxt")
                nc.gpsimd.partition_all_reduce(
                    mxt[:], xt[:], 128, bass_isa.ReduceOp.absmax
                )
                mx = small.tile([128, F], dt.float32, tag="mx")
                nc.vector.tensor_tensor(
                    out=mx[:], in0=mxt[:, 0:F], in1=mxt[:, F : 2 * F], op=op.max
                )
                eb = mx[:].bitcast(dt.int32)
                nc.vector.tensor_scalar(
                    out=eb, in0=mx[:].bitcast(dt.int32),
                    scalar1=_i32(0x7F800000), scalar2=None, op0=op.bitwise_and,
                )
                # db = bits(+delta) = value eb - 0x01000000 ; nr = bits(-recip)
                db = small.tile([128, F], dt.int32, tag="db")
                nr = small.tile([128, F], dt.int32, tag="nr")
                nc.scalar.activation(
                    out=db[:], in_=eb,
                    func=mybir.ActivationFunctionType.Identity,
                    bias=dbias[:], scale=1.0,
                )
                nc.scalar.mul(out=nr[:], in_=eb, mul=-1.0)
                nc.vector.tensor_scalar(
                    out=xt[:].bitcast(dt.int32), in0=xt[:].bitcast(dt.int32),
                    scalar1=1, scalar2=None, op0=op.bitwise_or,
                )
                # t = x2 * (-recip)  -> u = x2*recip = -t
                tt_ = mxt
                nrf = nr[:].bitcast(dt.float32)[:, None, :].broadcast_to([128, 2, F])
                x3 = xt[:].rearrange("p (r f) -> p r f", r=2)
                nc.vector.tensor_tensor(
                    out=tt_[:].rearrange("p (r f) -> p r f", r=2),
                    in0=x3, in1=nrf, op=op.mult,
                )
                # p = cvt(Relu(-t - c)) = trunc(u) for u>0 ; xt slot free
                p = xt[:].bitcast(dt.int32)
                nc.scalar.activation(
                    out=p, in_=tt_[:],
                    func=mybir.ActivationFunctionType.Relu,
                    bias=bias[:], scale=-1.0,
                )
                # nt = cvt(Relu(t - c)) = |trunc(u)| for u<0
                ntile = big.tile([128, 2 * F], dt.int32, tag="nt")
                nc.scalar.activation(
                    out=ntile[:], in_=tt_[:],
                    func=mybir.ActivationFunctionType.Relu,
                    bias=bias[:], scale=1.0,
                )
                # w = p - nt  (in place over tt_)
                w = tt_[:].bitcast(dt.int32)
                nc.vector.tensor_tensor(out=w, in0=p, in1=ntile[:], op=op.subtract)
                # q = w * delta  (over the nt slot)
                dbf = db[:].bitcast(dt.float32)[:, None, :].broadcast_to([128, 2, F])
                q3 = ntile[:].bitcast(dt.float32).rearrange("p (r f) -> p r f", r=2)
                nc.vector.tensor_tensor(
                    out=q3, in0=w.rearrange("p (r f) -> p r f", r=2),
                    in1=dbf, op=op.mult,
                )
                qf = ntile[:].bitcast(dt.float32)
                nc.scalar.dma_start(out=y_d[n, 0:128, :], in_=qf[:, 0:F])
                nc.scalar.dma_start(out=y_d[n, 128:256, :], in_=qf[:, F : 2 * F])
    nc.compile()
    _cache[("nc", repeat)] = nc
    return nc


def _run(x, trace=False, **kwargs):
    from concourse import bass_utils

    nc = _build()
    xs = np.ascontiguousarray(x.reshape(N, C, HW))
    in_maps = [
        {"x": xs[i * NPC : (i + 1) * NPC]} for i in range(NCORES)
    ]
    res = bass_utils.run_bass_kernel_spmd(
        nc, in_maps, core_ids=list(range(NCORES)), trace=trace, **kwargs
    )
    out = np.concatenate([r["y"] for r in res.results], axis=0)
    return out.reshape(N, C, H, W), res


def kernel(activations):
    out, _ = _run(np.asarray(activations))
    return out

